# revision 24
# baseline (speedup 1.0000x reference)
"""nn_GNNModel: 2-layer bipartite GraphSAGE + link classifier on 8 TRN2 cores.

Edge-parallel sharding by destination node: core c owns users [c*12500,
(c+1)*12500) and games [c*6250, (c+1)*6250). Each aggregation gathers
source-node feature rows (fp16, 256B) with gpsimd.dma_gather and reduces
them into PSUM via a scaled one-hot matmul (the mean's 1/deg folded into
the one-hot values). The dense SAGE transform runs at PSUM flush; outputs
are transposed back to row-major and AllGather-ed so the next layer can
gather from the full table. The link classifier is a pair of gathers plus
a fused multiply-reduce per 128 label edges.
"""

import numpy as np

P = 128
BANK = 32768
f16 = np.float16
f32 = np.float32


def _cdiv(a, b):
    return -(-a // b)


# ---------------------------------------------------------------- config


class CFG:
    def __init__(self, ncores, nu, ng, h, e, el,
                 maxi=1024, maxi_lk=1024, grp_u=2, grp_g=1, td=64):
        assert h == P
        self.ncores, self.nu, self.ng, self.h = ncores, nu, ng, h
        self.e, self.el = e, el
        self.td = td                                   # dst-tile width
        self.tpb = 512 // td                           # dst tiles per psum bank
        self.us, self.gs = nu // ncores, ng // ncores  # per-core shards
        self.tu = _cdiv(self.us, td)                   # user tiles / core
        self.tg = _cdiv(self.gs, td)                   # game tiles / core
        self.nup = ncores * self.tu * td               # padded user rows
        self.ngp = ncores * self.tg * td               # padded game rows
        self.maxi, self.maxi_lk = maxi, maxi_lk
        self.grp_u = grp_u * self.tpb              # tiles per gather group
        self.grp_g = grp_g * self.tpb


# ------------------------------------------------------- host-side prep


class Orient:
    """Core-independent slot layout for one aggregation orientation.

    Slots are grouped (group of `grp` dst tiles) -> (source bank) ->
    (dst tile), each (tile, bank) run padded to a multiple of 128 and
    sized to the max count over cores so a single NEFF fits all cores.
    """

    def __init__(self, cfg, dst, src, n_src_pad, shard, n_tiles, grp):
        td = cfg.td
        nb = _cdiv(n_src_pad, BANK)
        core = dst // shard
        tile = (dst - core * shard) // td
        bank = src >> 15
        key = (core * n_tiles + tile) * nb + bank
        counts = np.bincount(key, minlength=cfg.ncores * n_tiles * nb)
        counts = counts.reshape(cfg.ncores, n_tiles, nb)
        cap = _cdiv(counts.max(axis=0), P) * P         # [n_tiles, nb]
        cap[cap.sum(axis=1) == 0, 0] = P               # tile >=1 chunk
        self.nb, self.n_tiles, self.grp = nb, n_tiles, grp

        grp = cfg.tpb                                  # tiles per psum bank
        self.grp = grp
        rank = np.full((n_tiles, nb), -1, np.int64)
        run_off, run_cap = [], []
        off = 0
        for g in range(_cdiv(n_tiles, grp)):
            ts = range(g * grp, min((g + 1) * grp, n_tiles))
            for b in range(nb):
                for t in ts:
                    if cap[t, b]:
                        rank[t, b] = len(run_off)
                        run_off.append(off)
                        run_cap.append(int(cap[t, b]))
                        off += int(cap[t, b])
        self.rank = rank
        self.run_off = np.asarray(run_off, np.int64)
        self.slots = off
        self.chunks = off // P

        ct = np.empty(self.chunks, np.int64)
        for (t, b), idx in np.ndenumerate(rank):
            if idx >= 0:
                o = int(self.run_off[idx])
                ct[o // P:(o + int(cap[t, b])) // P] = t
        self.chunk_tile = ct
        self.tpb = cfg.tpb

        # gather instructions: (bank, slot_off, n), n <= maxi, 128-aligned;
        # grouped so all instrs of psum-group g are consecutive
        self.instrs = []
        self.group_instrs = []      # per group: list of instr indices
        self.group_tiles = []       # per group: list of tile ids
        for g in range(_cdiv(n_tiles, grp)):
            ts = range(g * grp, min((g + 1) * grp, n_tiles))
            self.group_tiles.append(list(ts))
            gi = []
            for b in range(nb):
                runs = [t for t in ts if cap[t, b]]
                if not runs:
                    continue
                start = int(self.run_off[rank[runs[0], b]])
                total = int(sum(cap[t, b] for t in runs))
                while total > 0:
                    take = min(total, cfg.maxi)
                    gi.append(len(self.instrs))
                    self.instrs.append((b, start, take))
                    start += take
                    total -= take
            self.group_instrs.append(gi)
        # chunk -> (instr idx, chunk-within-instr)
        self.chunk_instr = np.zeros((self.chunks, 2), np.int64)
        for i, (b, soff, n) in enumerate(self.instrs):
            for cl in range(n // P):
                self.chunk_instr[soff // P + cl] = (i, cl)
        # per group: emission order of chunks, tile-major then bank
        self.group_chunks = []      # per group: [(chunk, start, stop)]
        for g, ts in enumerate(self.group_tiles):
            order = []
            for t in ts:
                tcs = []
                for b in range(nb):
                    if rank[t, b] >= 0:
                        o = int(self.run_off[rank[t, b]])
                        tcs.extend(range(o // P, (o + int(cap[t, b])) // P))
                for j, c in enumerate(tcs):
                    order.append((c, j == 0, j == len(tcs) - 1))
            self.group_chunks.append(order)


def _fill_orient(cfg, o, dst, src, shard):
    """Per-core slot tables: int16 gather idx, fp16 dstloc.

    Pad slots keep dstloc=255, which never matches the iota (td<=128), so
    they drop out of the one-hot without a separate scale table. The mean's
    1/deg is applied per-dst at flush time instead.
    """
    core = dst // shard
    loc = dst - core * shard
    tile = loc // cfg.td
    dstloc = loc - tile * cfg.td
    lk = o.rank[tile, src >> 15]
    assert (lk >= 0).all()
    idx16 = np.zeros((cfg.ncores, o.slots), np.int16)
    dl = np.full((cfg.ncores, o.slots), 255, f16)
    for c in range(cfg.ncores):
        m = core == c
        lkc = lk[m]
        ordr = np.argsort(lkc, kind="stable")
        lks = lkc[ordr]
        cnts = np.bincount(lks, minlength=len(o.run_off))
        starts = np.concatenate([[0], np.cumsum(cnts)[:-1]])
        within = np.arange(len(lks)) - starts[lks]
        slots = o.run_off[lks] + within
        idx16[c, slots] = (src[m][ordr] & (BANK - 1)).astype(np.int16)
        dl[c, slots] = dstloc[m][ordr].astype(f16)
    return idx16, dl


def _pack_idx(a):
    """[SLOTS] int16 -> [128, SLOTS//16]: slot j -> [j%16, j//16], x8."""
    n = a.shape[-1]
    w = np.swapaxes(a.reshape(n // 16, 16), -1, -2)
    return np.ascontiguousarray(np.tile(w, (8, 1)))


def _pack_cols(a):
    """[SLOTS] -> [128, SLOTS//128]: slot j -> [j%128, j//128]."""
    n = a.shape[-1]
    return np.ascontiguousarray(np.swapaxes(a.reshape(n // P, P), -1, -2))


class LinkPrep:
    def __init__(self, cfg, el_src, el_dst):
        nbg = _cdiv(cfg.ngp, BANK)
        core = el_src // cfg.us
        bank = el_dst >> 15
        counts = np.bincount(core * nbg + bank, minlength=cfg.ncores * nbg)
        capl = _cdiv(counts.reshape(cfg.ncores, nbg).max(axis=0), P) * P
        np.maximum(capl, P, out=capl)
        self.nbg = nbg
        self.off = np.concatenate([[0], np.cumsum(capl)])
        self.slots = int(self.off[-1])
        self.chunks = self.slots // P
        self.instrs_g = []
        for b in range(nbg):
            start, total = int(self.off[b]), int(capl[b])
            while total > 0:
                take = min(total, cfg.maxi_lk)
                self.instrs_g.append((b, start, take))
                start += take
                total -= take
        self.instrs_u = []
        start, total = 0, self.slots
        while total > 0:
            take = min(total, cfg.maxi_lk)
            self.instrs_u.append((0, start, take))
            start += take
            total -= take

        self.uidx = np.zeros((cfg.ncores, self.slots), np.int16)
        self.gidx = np.zeros((cfg.ncores, self.slots), np.int16)
        self.orig = np.full((cfg.ncores, self.slots), -1, np.int64)
        for c in range(cfg.ncores):
            m = core == c
            es, ed = el_src[m], el_dst[m]
            oi = np.nonzero(m)[0]
            b = ed >> 15
            ordr = np.argsort(b, kind="stable")
            bs_ = b[ordr]
            cnts = np.bincount(bs_, minlength=nbg)
            starts = np.concatenate([[0], np.cumsum(cnts)[:-1]])
            within = np.arange(len(bs_)) - starts[bs_]
            slots = self.off[bs_] + within
            self.uidx[c, slots] = (es[ordr] - c * cfg.us).astype(np.int16)
            self.gidx[c, slots] = (ed[ordr] & (BANK - 1)).astype(np.int16)
            self.orig[c, slots] = oi[ordr]


def prep(cfg, edge_src, edge_dst, el_src, el_dst):
    su = 1.0 / np.maximum(
        np.bincount(edge_src, minlength=cfg.nu).astype(f32), 1.0)
    sg = 1.0 / np.maximum(
        np.bincount(edge_dst, minlength=cfg.ng).astype(f32), 1.0)
    # per-core, per-local-dst recip tables, padded rows -> 1.0
    rcp_u = np.ones((cfg.ncores, cfg.tu * cfg.td), f16)
    rcp_g = np.ones((cfg.ncores, cfg.tg * cfg.td), f16)
    for c in range(cfg.ncores):
        rcp_u[c, : cfg.us] = su[c * cfg.us: (c + 1) * cfg.us]
        rcp_g[c, : cfg.gs] = sg[c * cfg.gs: (c + 1) * cfg.gs]
    # user-agg: dst=edge_src (users), src=edge_dst (games)
    ua = Orient(cfg, edge_src, edge_dst, cfg.ngp, cfg.us, cfg.tu, cfg.grp_u)
    ua_t = _fill_orient(cfg, ua, edge_src, edge_dst, cfg.us)
    # game-agg: dst=edge_dst (games), src=edge_src (users)
    ga = Orient(cfg, edge_dst, edge_src, cfg.nup, cfg.gs, cfg.tg, cfg.grp_g)
    ga_t = _fill_orient(cfg, ga, edge_dst, edge_src, cfg.gs)
    lk = LinkPrep(cfg, el_src, el_dst)
    return ua, ua_t, ga, ga_t, lk, (rcp_u, rcp_g)


# ------------------------------------------------------- device program


def build(cfg, ua, ga, lk):
    import os as _os
    KO = _os.environ.get("GNN_KO", "").split(",")
    import concourse.mybir as mybir
    import concourse.tile as tile
    from concourse.tile import add_dep_helper
    from concourse import bacc
    from concourse.bass import BassGpSimd

    dt16 = mybir.dt.float16
    dt32 = mybir.dt.float32
    dti = mybir.dt.int16
    alu = mybir.AluOpType
    act = mybir.ActivationFunctionType
    N = cfg.ncores

    NQ = 4                       # SWDGE queues: gathers on different queues
    nc = bacc.Bacc(None, target_bir_lowering=True, num_devices=N,
                   num_swdge_queues=NQ,
                   dynamic_dma_scratch_size=32768)

    def inp(name, shape, dt):
        return nc.dram_tensor(name, shape, dt, kind="ExternalInput")

    xu = inp("xu", [cfg.nup, P], dt16)            # full x_user (replicated)
    xg = inp("xg", [cfg.ngp, P], dt16)            # full x_game (replicated)
    xu_s = inp("xu_s", [cfg.tu * cfg.td, P], dt16)     # this core's user shard
    xg_s = inp("xg_s", [cfg.tg * cfg.td, P], dt16)     # this core's game shard
    iota = inp("iota", [P, P], dt16)
    ident = inp("ident", [P, P], dt16)
    ws = {k: inp(k, [P, P], dt16)
          for k in ("wl1u", "wr1u", "wl1g", "wr1g",
                    "wl2u", "wr2u", "wl2g", "wr2g")}
    bs = {k: inp(k, [P, 1], dt32) for k in ("b1u", "b1g", "b2u", "b2g")}
    ua_idx = inp("ua_idx", [P, ua.slots // 16], dti)
    ua_dl = inp("ua_dl", [P, ua.chunks], dt16)
    ga_idx = inp("ga_idx", [P, ga.slots // 16], dti)
    ga_dl = inp("ga_dl", [P, ga.chunks], dt16)
    rcpu = inp("rcpu", [P, cfg.tu * cfg.td], dt16)
    rcpg = inp("rcpg", [P, cfg.tg * cfg.td], dt16)
    lk_ui = inp("lk_ui", [P, lk.slots // 16], dti)
    lk_gi = inp("lk_gi", [P, lk.slots // 16], dti)
    out = nc.dram_tensor("out", [P, lk.chunks], dt32, kind="ExternalOutput")
    dbg = nc.dram_tensor("dbg", [cfg.tu * cfg.td, P], dt16,
                         kind="ExternalOutput") if _os.environ.get("GNN_DBG") \
        else None
    dbgm = nc.dram_tensor("dbgm", [P, cfg.tu * cfg.td], dt16,
                          kind="ExternalOutput") \
        if _os.environ.get("GNN_DBG") == "mean" else None

    u_sh = nc.dram_tensor("u_sh", [cfg.tu * cfg.td, P], dt16)
    g_sh = nc.dram_tensor("g_sh", [cfg.tg * cfg.td, P], dt16)
    u2_sh = nc.dram_tensor("u2_sh", [cfg.tu * cfg.td, P], dt16)
    g2_sh = nc.dram_tensor("g2_sh", [cfg.gs, P], dt16)
    u_full = nc.dram_tensor("u_full", [N * cfg.us, P], dt16,
                            addr_space="Shared")
    g_full = nc.dram_tensor("g_full", [N * cfg.gs, P], dt16,
                            addr_space="Shared")
    g2_full = nc.dram_tensor("g2_full", [N * cfg.gs, P], dt16,
                             addr_space="Shared")
    rg = [list(range(N))]
    gchain = {q: [None, None] for q in range(NQ)}
    qrr = [0]

    def next_q():
        q = qrr[0]
        qrr[0] = (q + 1) % NQ
        return q

    def throttle(gi_inst, q):
        # dma_gather ucode doesn't check SWDGE ring space: keep <=2 gathers
        # (<=2048 descriptors) outstanding per queue by chaining to the
        # queue's 2nd-previous.
        ch = gchain[q]
        if ch[0] is not None:
            add_dep_helper(gi_inst.ins, ch[0],
                           reason="swdge ring throttle")
        ch[0], ch[1] = ch[1], gi_inst.ins

    def allgather(in_ap, out_ap):
        # Run the collective on the (otherwise idle) Scalar queue so its
        # network time overlaps the Pool gather stream instead of blocking
        # it; consumers wait on the cc's completion via tile deps.
        if "cc" in KO:
            return
        # walrus' BIR verifier pins InstCollectiveCompute to the Pool engine
        nc.gpsimd.collective_compute(
            "AllGather", alu.bypass, replica_groups=rg,
            ins=[in_ap], outs=[out_ap])

    with tile.TileContext(nc) as tc:
        with (
            tc.tile_pool(name="res", bufs=1) as res,
            tc.tile_pool(name="gst", bufs=20) as gst,
            tc.tile_pool(name="osb", bufs=20) as osb,
            tc.tile_pool(name="flp", bufs=5) as flp,
            tc.tile_pool(name="psA", bufs=4, space="PSUM") as psA,
            tc.tile_pool(name="psB", bufs=2, space="PSUM") as psB,
            tc.tile_pool(name="lkp", bufs=4) as lkp,
        ):
            def resident(dram, shape, dt, tag):
                t = res.tile(shape, dt, tag=tag)
                nc.sync.dma_start(t[:], dram[:])
                return t

            iota_sb = res.tile([P, 1, P], dt16, tag="iota", name="iota_sb")
            nc.sync.dma_start(iota_sb[:, 0, :], iota[:])
            id_sb = resident(ident, [P, P], dt16, "ident")
            w_sb = {k: resident(v, [P, P], dt16, k) for k, v in ws.items()}
            b_sb = {k: resident(v, [P, 1], dt32, k) for k, v in bs.items()}
            uai = resident(ua_idx, [P, ua.slots // 16], dti, "uai")
            uad = resident(ua_dl, [P, ua.chunks], dt16, "uad")
            gai = resident(ga_idx, [P, ga.slots // 16], dti, "gai")
            gad = resident(ga_dl, [P, ga.chunks], dt16, "gad")
            lui = resident(lk_ui, [P, lk.slots // 16], dti, "lui")
            lgi = resident(lk_gi, [P, lk.slots // 16], dti, "lgi")

            def agg_layer(o, idx_sb, dl_sb, rcp_dram, src_dram, src_rows,
                          xsrc_dram, n_tiles, wl, wr, b, relu,
                          out_dram, out_rows):
                def flush(g, pt):
                    r0 = g * 512
                    w = min(512, n_tiles * cfg.td - r0)
                    rt = flp.tile([P, 512], dt16, tag="rcp")
                    nc.scalar.dma_start(out=rt[:, :w],
                                        in_=rcp_dram[:, r0: r0 + w])
                    mean_sb = flp.tile([P, 512], dt16, tag="mean")
                    if "mm" in KO:
                        nc.vector.memset(mean_sb[:, :w], 0.0)
                    else:
                        nc.vector.tensor_tensor(
                            out=mean_sb[:, :w], in0=pt[:, :w],
                            in1=rt[:, :w], op=alu.mult)
                    if dbgm is not None and out_dram is u_sh:
                        nc.sync.dma_start(dbgm[:, r0: r0 + w],
                                          mean_sb[:, :w])
                    xt = flp.tile([P, 512], dt16, tag="xt")
                    if "xt" in KO:
                        nc.vector.memset(xt[:, :w], 0.0)
                    else:
                        nc.sync.dma_start(
                            out=xt[:, :w],
                            in_=xsrc_dram[r0: r0 + w, :],
                            transpose=True,
                        )
                    ps2 = psB.tile([P, 512], dt32, tag="psB")
                    nc.tensor.matmul(ps2[:, :w], lhsT=wl[:],
                                     rhs=mean_sb[:, :w],
                                     start=True, stop=False)
                    nc.tensor.matmul(ps2[:, :w], lhsT=wr[:], rhs=xt[:, :w],
                                     start=False, stop=True)
                    # bias+relu on DVE (keeps the Scalar queue free for the
                    # collectives, which must not stall behind flush work)
                    ot = flp.tile([P, 512], dt16, tag="ot")
                    nc.vector.tensor_tensor(
                        out=ot[:, :w], in0=ps2[:, :w],
                        in1=b[:].to_broadcast([P, w]), op=alu.add)
                    if relu:
                        nc.vector.tensor_scalar_max(ot[:, :w], ot[:, :w],
                                                    0.0)
                    ps3 = psB.tile([P, 512], dt16, tag="psB3")
                    ut = flp.tile([P, 512], dt16, tag="ut")
                    if "tr" in KO:
                        nc.vector.tensor_copy(ut[:, :w], ot[:, :w])
                    else:
                        for k in range(w // P):
                            nc.tensor.transpose(
                                ps3[:, k * P:(k + 1) * P],
                                ot[:, k * P:(k + 1) * P], id_sb[:])
                        nc.vector.tensor_copy(ut[:, :w], ps3[:, :w])
                    if r0 + w <= out_rows and w % P == 0:
                        nc.sync.dma_start(
                            out=out_dram[r0: r0 + w, :].rearrange(
                                "(k p) h -> p k h", p=P),
                            in_=ut[:, :w].rearrange("p (k h) -> p k h", h=P),
                        )
                    else:
                        for k in range(w // P):
                            rk = r0 + k * P
                            rw = min(P, out_rows - rk)
                            if rw <= 0:
                                break
                            nc.sync.dma_start(out=out_dram[rk: rk + rw, :],
                                              in_=ut[:rw, k * P:(k + 1) * P])

                td = cfg.td
                for g, gi in enumerate(o.group_instrs):
                    stages = {}
                    ohws = {}
                    for i in gi:
                        bnk, soff, n = o.instrs[i]
                        stage = gst.tile([P, cfg.maxi // P, P], dt16,
                                         tag="gst")
                        roff = bnk * BANK
                        rows = min(BANK, src_rows - roff)
                        if "gat" in KO:
                            nc.vector.memset(stage[:, : n // P, :], 0.0)
                        else:
                            q = next_q()
                            throttle(nc.gpsimd.dma_gather(
                                out_ap=stage[:, : n // P, :],
                                in_ap=src_dram[roff: roff + rows, :],
                                idxs_ap=idx_sb[:, soff // 16:
                                               (soff + n) // 16],
                                num_idxs=n,
                                num_idxs_reg=n,
                                elem_size=P,
                                queue_num=q,
                            ), q)
                        stages[i] = stage
                        if "mm" in KO:
                            continue
                        nk = n // P
                        c0 = soff // P
                        ohw = osb.tile([P, cfg.maxi // P, td], dt16,
                                       tag="ohw")
                        nc.vector.tensor_tensor(
                            out=ohw[:, :nk, :],
                            in0=iota_sb[:, :1, :td].to_broadcast(
                                [P, nk, td]),
                            in1=dl_sb[:, c0: c0 + nk].to_broadcast(
                                [P, nk, td]),
                            op=alu.is_equal,
                        )
                        ohws[i] = ohw
                    pt = psA.tile([P, 512], dt32, tag="psA", name="psA_t")
                    if "mm" not in KO:
                        for (c, first, last) in o.group_chunks[g]:
                            i, cl = (int(o.chunk_instr[c, 0]),
                                     int(o.chunk_instr[c, 1]))
                            t = int(o.chunk_tile[c])
                            col = (t % cfg.tpb) * td
                            nc.tensor.matmul(
                                pt[:, col: col + td],
                                lhsT=stages[i][:, cl, :],
                                rhs=ohws[i][:, cl, :],
                                start=bool(first), stop=bool(last),
                            )
                    flush(g, pt)

            # zero the pad rows that L2 transpose-loads will read
            zt = res.tile([P, P], dt16, tag="zero")
            nc.gpsimd.memset(zt[:], 0.0)
            if cfg.tu * cfg.td > cfg.us:
                nc.sync.dma_start(out=u_sh[cfg.us: cfg.tu * cfg.td, :],
                                  in_=zt[: cfg.tu * cfg.td - cfg.us, :])
            if cfg.tg * cfg.td > cfg.gs:
                nc.sync.dma_start(out=g_sh[cfg.gs: cfg.tg * cfg.td, :],
                                  in_=zt[: cfg.tg * cfg.td - cfg.gs, :])

            # ---- layer 1
            agg_layer(ua, uai, uad, rcpu, xg, cfg.ngp, xu_s, cfg.tu,
                      w_sb["wl1u"], w_sb["wr1u"], b_sb["b1u"], True,
                      u_sh, cfg.us)
            allgather(u_sh[: cfg.us, :].opt(), u_full[:].opt())
            agg_layer(ga, gai, gad, rcpg, xu, cfg.nup, xg_s, cfg.tg,
                      w_sb["wl1g"], w_sb["wr1g"], b_sb["b1g"], True,
                      g_sh, cfg.gs)
            allgather(g_sh[: cfg.gs, :].opt(), g_full[:].opt())

            # ---- layer 2 (game side first: it only needs AG_u)
            agg_layer(ga, gai, gad, rcpg, u_full, N * cfg.us, g_sh, cfg.tg,
                      w_sb["wl2g"], w_sb["wr2g"], b_sb["b2g"], False,
                      g2_sh, cfg.gs)
            allgather(g2_sh[:].opt(), g2_full[:].opt())
            agg_layer(ua, uai, uad, rcpu, g_full, N * cfg.gs, u_sh, cfg.tu,
                      w_sb["wl2u"], w_sb["wr2u"], b_sb["b2u"], False,
                      u2_sh, cfg.us)

            # ---- link classifier: interleave u/g gathers with consumption
            ost = res.tile([P, lk.chunks], dt32, tag="ost")
            if "lkg" in KO:
                nc.vector.memset(ost[:], 0.0)
            iu = ig = 0
            cur = {}

            def maybe_gather(side, instrs, i, idxsb, src, rows):
                if i >= len(instrs):
                    return i
                bnk, soff, n = instrs[i]
                stage = lkp.tile([P, cfg.maxi_lk // P, P], dt16,
                                 tag="lk" + side)
                roff = bnk * BANK
                rw = min(BANK, rows - roff)
                if "lgo" in KO:
                    nc.vector.memset(stage[:, : n // P, :], 0.0)
                else:
                    q = next_q()
                    throttle(nc.gpsimd.dma_gather(
                        out_ap=stage[:, : n // P, :],
                        in_ap=src[roff: roff + rw, :],
                        idxs_ap=idxsb[:, soff // 16: (soff + n) // 16],
                        num_idxs=n, num_idxs_reg=n, elem_size=P,
                        queue_num=q,
                    ), q)
                cur[side] = (stage, soff // P, soff // P + n // P)
                return i + 1

            for c in range(lk.chunks if "lkg" not in KO else 0):
                if "u" not in cur or c >= cur["u"][2]:
                    iu = maybe_gather("u", lk.instrs_u, iu, lui, u2_sh,
                                      cfg.tu * cfg.td)
                if "g" not in cur or c >= cur["g"][2]:
                    ig = maybe_gather("g", lk.instrs_g, ig, lgi, g2_full,
                                      N * cfg.gs)
                sut, u0, _ = cur["u"]
                sgt, g0, _ = cur["g"]
                scr = lkp.tile([P, P], dt32, tag="lscr")
                nc.vector.tensor_tensor(
                    out=scr[:],
                    in0=sut[:, c - u0, :],
                    in1=sgt[:, c - g0, :],
                    op=alu.mult,
                )
                nc.vector.tensor_reduce(
                    out=ost[:, c: c + 1],
                    in_=scr[:],
                    axis=mybir.AxisListType.X,
                    op=alu.add,
                )
            nc.sync.dma_start(out=out[:], in_=ost[:])
            if dbg is not None:
                dt = res.tile([P, 512], dt16, tag="dbgt")
                for r0 in range(0, cfg.tu * cfg.td, 512):
                    w = min(512, cfg.tu * cfg.td - r0)
                    src = {"u": u_sh, "g": g_sh, "u2": u2_sh,
                           "mean": u_sh}[_os.environ["GNN_DBG"]]
                    nc.sync.dma_start(dt[:, :w].rearrange(
                        "p (k h) -> p k h", h=P)[:, : w // P, :],
                        src[r0: r0 + w, :].rearrange(
                            "(k p) h -> p k h", p=P))
                    nc.sync.dma_start(
                        dbg[r0: r0 + w, :].rearrange("(k p) h -> p k h", p=P),
                        dt[:, :w].rearrange("p (k h) -> p k h", h=P)[
                            :, : w // P, :])


    nc.compile()
    return nc


# ------------------------------------------------------------- kernel()


def _make_inputs(cfg, xu_np, xg_np, wmap, bmap, ua_t, ga_t, lk, rcp):
    iota_np = np.tile(np.arange(P, dtype=f16), (P, 1))
    id_np = np.eye(P, dtype=f16)
    ua_i, ua_d = ua_t
    ga_i, ga_d = ga_t
    rcp_u, rcp_g = rcp
    in_maps = []
    for c in range(cfg.ncores):
        m = {"xu": xu_np, "xg": xg_np, "iota": iota_np, "ident": id_np,
             "xu_s": xu_np[c * cfg.us: c * cfg.us + cfg.tu * cfg.td],
             "xg_s": xg_np[c * cfg.gs: c * cfg.gs + cfg.tg * cfg.td]}
        for k, v in wmap.items():
            m[k] = np.asarray(v, f32).astype(f16)
        for k, v in bmap.items():
            m[k] = np.asarray(v, f32).reshape(P, 1)
        m["ua_idx"] = _pack_idx(ua_i[c])
        m["ua_dl"] = _pack_cols(ua_d[c])
        m["ga_idx"] = _pack_idx(ga_i[c])
        m["ga_dl"] = _pack_cols(ga_d[c])
        m["rcpu"] = np.ascontiguousarray(np.tile(rcp_u[c], (P, 1)))
        m["rcpg"] = np.ascontiguousarray(np.tile(rcp_g[c], (P, 1)))
        m["lk_ui"] = _pack_idx(lk.uidx[c])
        m["lk_gi"] = _pack_idx(lk.gidx[c])
        in_maps.append(m)
    return in_maps


_cache = {}


def run(cfg, inputs_np, run_kwargs=None):
    """Full pipeline given reference-named inputs; returns [EL] fp32."""
    from concourse.bass_utils import run_bass_kernel_spmd

    ii = inputs_np
    edge_src = np.asarray(ii["edge_src"], np.int64)
    edge_dst = np.asarray(ii["edge_dst"], np.int64)
    el_src = np.asarray(ii["el_src"], np.int64)
    el_dst = np.asarray(ii["el_dst"], np.int64)

    xu_np = np.zeros((cfg.nup, P), f16)
    xu_np[: cfg.nu] = np.asarray(ii["user_emb"], f32)[
        np.asarray(ii["user_node_id"])]
    xg_np = np.zeros((cfg.ngp, P), f16)
    xg_np[: cfg.ng] = (
        np.asarray(ii["game_x"], f32) @ np.asarray(ii["game_lin_w"], f32)
        + np.asarray(ii["game_lin_b"], f32)
        + np.asarray(ii["game_emb"], f32)[np.asarray(ii["game_node_id"])])

    key = (edge_src[:64].tobytes(), edge_dst[:64].tobytes(),
           int(edge_src.sum()), int(edge_dst.sum()))
    if key in _cache:
        ua, ua_t, ga, ga_t, lk, rcp, nc = _cache[key]
    else:
        ua, ua_t, ga, ga_t, lk, rcp = prep(cfg, edge_src, edge_dst,
                                           el_src, el_dst)
        nc = build(cfg, ua, ga, lk)
        _cache[key] = (ua, ua_t, ga, ga_t, lk, rcp, nc)

    wmap = {"wl1u": ii["w1l_r"], "wr1u": ii["w1r_r"],
            "wl1g": ii["w1l_o"], "wr1g": ii["w1r_o"],
            "wl2u": ii["w2l_r"], "wr2u": ii["w2r_r"],
            "wl2g": ii["w2l_o"], "wr2g": ii["w2r_o"]}
    bmap = {"b1u": ii["b1l_r"], "b1g": ii["b1l_o"],
            "b2u": ii["b2l_r"], "b2g": ii["b2l_o"]}
    in_maps = _make_inputs(cfg, xu_np, xg_np, wmap, bmap, ua_t, ga_t, lk,
                           rcp)

    res = run_bass_kernel_spmd(nc, in_maps,
                               core_ids=list(range(cfg.ncores)),
                               **(run_kwargs or {}))
    global LAST_RESULTS
    LAST_RESULTS = res

    result = np.zeros(cfg.el, f32)
    for c in range(cfg.ncores):
        oc = np.asarray(res.results[c]["out"], f32)   # [128, chunks]
        flat = oc.T.reshape(-1)                        # slot j = flat[j]
        valid = lk.orig[c] >= 0
        result[lk.orig[c][valid]] = flat[valid]
    return result


def kernel(user_node_id, game_node_id, game_x, edge_src, edge_dst,
           el_src, el_dst, user_emb, game_emb, game_lin_w, game_lin_b,
           w1l_o, b1l_o, w1r_o, w1l_r, b1l_r, w1r_r,
           w2l_o, b2l_o, w2r_o, w2l_r, b2l_r, w2r_r):
    cfg = CFG(8, 100000, 50000, 128, 1000000, 500000)
    return run(cfg, dict(
        user_node_id=user_node_id, game_node_id=game_node_id,
        game_x=game_x, edge_src=edge_src, edge_dst=edge_dst,
        el_src=el_src, el_dst=el_dst, user_emb=user_emb,
        game_emb=game_emb, game_lin_w=game_lin_w, game_lin_b=game_lin_b,
        w1l_o=w1l_o, b1l_o=b1l_o, w1r_o=w1r_o,
        w1l_r=w1l_r, b1l_r=b1l_r, w1r_r=w1r_r,
        w2l_o=w2l_o, b2l_o=b2l_o, w2r_o=w2r_o,
        w2l_r=w2l_r, b2l_r=b2l_r, w2r_r=w2r_r))



# revision 26
# speedup vs baseline: 1.1838x; 1.1838x over previous
"""nn_GNNModel: 2-layer bipartite GraphSAGE + link classifier on 8 TRN2 cores.

Edge-parallel sharding by destination node: core c owns users [c*12500,
(c+1)*12500) and games [c*6250, (c+1)*6250). Each aggregation gathers
source-node feature rows (fp16, 256B) with gpsimd.dma_gather and reduces
them into PSUM via a scaled one-hot matmul (the mean's 1/deg folded into
the one-hot values). The dense SAGE transform runs at PSUM flush; outputs
are transposed back to row-major and AllGather-ed so the next layer can
gather from the full table. The link classifier is a pair of gathers plus
a fused multiply-reduce per 128 label edges.
"""

import numpy as np

P = 128
BANK = 32768
f16 = np.float16
f32 = np.float32


def _cdiv(a, b):
    return -(-a // b)


# ---------------------------------------------------------------- config


class CFG:
    def __init__(self, ncores, nu, ng, h, e, el,
                 maxi=1024, maxi_lk=1024, grp_u=2, grp_g=1, td=64):
        assert h == P
        self.ncores, self.nu, self.ng, self.h = ncores, nu, ng, h
        self.e, self.el = e, el
        self.td = td                                   # dst-tile width
        self.tpb = 512 // td                           # dst tiles per psum bank
        self.us, self.gs = nu // ncores, ng // ncores  # per-core shards
        self.tu = _cdiv(self.us, td)                   # user tiles / core
        self.tg = _cdiv(self.gs, td)                   # game tiles / core
        self.nup = ncores * self.tu * td               # padded user rows
        self.ngp = ncores * self.tg * td               # padded game rows
        self.maxi, self.maxi_lk = maxi, maxi_lk
        self.grp_u = grp_u * self.tpb              # tiles per gather group
        self.grp_g = grp_g * self.tpb


# ------------------------------------------------------- host-side prep


class Orient:
    """Core-independent slot layout for one aggregation orientation.

    Slots are grouped (group of `grp` dst tiles) -> (source bank) ->
    (dst tile), each (tile, bank) run padded to a multiple of 128 and
    sized to the max count over cores so a single NEFF fits all cores.
    """

    def __init__(self, cfg, dst, src, n_src_pad, shard, n_tiles, grp):
        td = cfg.td
        nb = _cdiv(n_src_pad, BANK)
        core = dst // shard
        tile = (dst - core * shard) // td
        bank = src >> 15
        key = (core * n_tiles + tile) * nb + bank
        counts = np.bincount(key, minlength=cfg.ncores * n_tiles * nb)
        counts = counts.reshape(cfg.ncores, n_tiles, nb)
        cap = _cdiv(counts.max(axis=0), P) * P         # [n_tiles, nb]
        cap[cap.sum(axis=1) == 0, 0] = P               # tile >=1 chunk
        self.nb, self.n_tiles, self.grp = nb, n_tiles, grp

        grp = cfg.tpb                                  # tiles per psum bank
        self.grp = grp
        rank = np.full((n_tiles, nb), -1, np.int64)
        run_off, run_cap = [], []
        off = 0
        for g in range(_cdiv(n_tiles, grp)):
            ts = range(g * grp, min((g + 1) * grp, n_tiles))
            for b in range(nb):
                for t in ts:
                    if cap[t, b]:
                        rank[t, b] = len(run_off)
                        run_off.append(off)
                        run_cap.append(int(cap[t, b]))
                        off += int(cap[t, b])
        self.rank = rank
        self.run_off = np.asarray(run_off, np.int64)
        self.slots = off
        self.chunks = off // P

        ct = np.empty(self.chunks, np.int64)
        for (t, b), idx in np.ndenumerate(rank):
            if idx >= 0:
                o = int(self.run_off[idx])
                ct[o // P:(o + int(cap[t, b])) // P] = t
        self.chunk_tile = ct
        self.tpb = cfg.tpb

        # gather instructions: (bank, slot_off, n), n <= maxi, 128-aligned;
        # grouped so all instrs of psum-group g are consecutive
        self.instrs = []
        self.group_instrs = []      # per group: list of instr indices
        self.group_tiles = []       # per group: list of tile ids
        for g in range(_cdiv(n_tiles, grp)):
            ts = range(g * grp, min((g + 1) * grp, n_tiles))
            self.group_tiles.append(list(ts))
            gi = []
            for b in range(nb):
                runs = [t for t in ts if cap[t, b]]
                if not runs:
                    continue
                start = int(self.run_off[rank[runs[0], b]])
                total = int(sum(cap[t, b] for t in runs))
                while total > 0:
                    take = min(total, cfg.maxi)
                    gi.append(len(self.instrs))
                    self.instrs.append((b, start, take))
                    start += take
                    total -= take
            self.group_instrs.append(gi)
        # chunk -> (instr idx, chunk-within-instr)
        self.chunk_instr = np.zeros((self.chunks, 2), np.int64)
        for i, (b, soff, n) in enumerate(self.instrs):
            for cl in range(n // P):
                self.chunk_instr[soff // P + cl] = (i, cl)
        # per group: emission order of chunks, tile-major then bank
        self.group_chunks = []      # per group: [(chunk, start, stop)]
        for g, ts in enumerate(self.group_tiles):
            order = []
            for t in ts:
                tcs = []
                for b in range(nb):
                    if rank[t, b] >= 0:
                        o = int(self.run_off[rank[t, b]])
                        tcs.extend(range(o // P, (o + int(cap[t, b])) // P))
                for j, c in enumerate(tcs):
                    order.append((c, j == 0, j == len(tcs) - 1))
            self.group_chunks.append(order)


def _fill_orient(cfg, o, dst, src, shard):
    """Per-core slot tables: int16 gather idx, fp16 dstloc.

    Pad slots keep dstloc=255, which never matches the iota (td<=128), so
    they drop out of the one-hot without a separate scale table. The mean's
    1/deg is applied per-dst at flush time instead.
    """
    core = dst // shard
    loc = dst - core * shard
    tile = loc // cfg.td
    dstloc = loc - tile * cfg.td
    lk = o.rank[tile, src >> 15]
    assert (lk >= 0).all()
    idx16 = np.zeros((cfg.ncores, o.slots), np.int16)
    dl = np.full((cfg.ncores, o.slots), 255, f16)
    for c in range(cfg.ncores):
        m = core == c
        lkc = lk[m]
        # secondary sort by src row: HBM row-buffer locality in the gathers
        ordr = np.lexsort((src[m], lkc))
        lks = lkc[ordr]
        cnts = np.bincount(lks, minlength=len(o.run_off))
        starts = np.concatenate([[0], np.cumsum(cnts)[:-1]])
        within = np.arange(len(lks)) - starts[lks]
        slots = o.run_off[lks] + within
        idx16[c, slots] = (src[m][ordr] & (BANK - 1)).astype(np.int16)
        dl[c, slots] = dstloc[m][ordr].astype(f16)
    return idx16, dl


def _pack_idx(a):
    """[SLOTS] int16 -> [128, SLOTS//16]: slot j -> [j%16, j//16], x8."""
    n = a.shape[-1]
    w = np.swapaxes(a.reshape(n // 16, 16), -1, -2)
    return np.ascontiguousarray(np.tile(w, (8, 1)))


def _pack_cols(a):
    """[SLOTS] -> [128, SLOTS//128]: slot j -> [j%128, j//128]."""
    n = a.shape[-1]
    return np.ascontiguousarray(np.swapaxes(a.reshape(n // P, P), -1, -2))


class LinkPrep:
    def __init__(self, cfg, el_src, el_dst):
        nbg = _cdiv(cfg.ngp, BANK)
        core = el_src // cfg.us
        bank = el_dst >> 15
        counts = np.bincount(core * nbg + bank, minlength=cfg.ncores * nbg)
        capl = _cdiv(counts.reshape(cfg.ncores, nbg).max(axis=0), P) * P
        np.maximum(capl, P, out=capl)
        self.nbg = nbg
        self.off = np.concatenate([[0], np.cumsum(capl)])
        self.slots = int(self.off[-1])
        self.chunks = self.slots // P
        self.instrs_g = []
        for b in range(nbg):
            start, total = int(self.off[b]), int(capl[b])
            while total > 0:
                take = min(total, cfg.maxi_lk)
                self.instrs_g.append((b, start, take))
                start += take
                total -= take
        self.instrs_u = []
        start, total = 0, self.slots
        while total > 0:
            take = min(total, cfg.maxi_lk)
            self.instrs_u.append((0, start, take))
            start += take
            total -= take

        self.uidx = np.zeros((cfg.ncores, self.slots), np.int16)
        self.gidx = np.zeros((cfg.ncores, self.slots), np.int16)
        self.orig = np.full((cfg.ncores, self.slots), -1, np.int64)
        for c in range(cfg.ncores):
            m = core == c
            es, ed = el_src[m], el_dst[m]
            oi = np.nonzero(m)[0]
            b = ed >> 15
            ordr = np.argsort(b, kind="stable")
            bs_ = b[ordr]
            cnts = np.bincount(bs_, minlength=nbg)
            starts = np.concatenate([[0], np.cumsum(cnts)[:-1]])
            within = np.arange(len(bs_)) - starts[bs_]
            slots = self.off[bs_] + within
            self.uidx[c, slots] = (es[ordr] - c * cfg.us).astype(np.int16)
            self.gidx[c, slots] = (ed[ordr] & (BANK - 1)).astype(np.int16)
            self.orig[c, slots] = oi[ordr]


def prep(cfg, edge_src, edge_dst, el_src, el_dst):
    su = 1.0 / np.maximum(
        np.bincount(edge_src, minlength=cfg.nu).astype(f32), 1.0)
    sg = 1.0 / np.maximum(
        np.bincount(edge_dst, minlength=cfg.ng).astype(f32), 1.0)
    # per-core, per-local-dst recip tables, padded rows -> 1.0
    rcp_u = np.ones((cfg.ncores, cfg.tu * cfg.td), f16)
    rcp_g = np.ones((cfg.ncores, cfg.tg * cfg.td), f16)
    for c in range(cfg.ncores):
        rcp_u[c, : cfg.us] = su[c * cfg.us: (c + 1) * cfg.us]
        rcp_g[c, : cfg.gs] = sg[c * cfg.gs: (c + 1) * cfg.gs]
    # user-agg: dst=edge_src (users), src=edge_dst (games)
    ua = Orient(cfg, edge_src, edge_dst, cfg.ngp, cfg.us, cfg.tu, cfg.grp_u)
    ua_t = _fill_orient(cfg, ua, edge_src, edge_dst, cfg.us)
    # game-agg: dst=edge_dst (games), src=edge_src (users)
    ga = Orient(cfg, edge_dst, edge_src, cfg.nup, cfg.gs, cfg.tg, cfg.grp_g)
    ga_t = _fill_orient(cfg, ga, edge_dst, edge_src, cfg.gs)
    lk = LinkPrep(cfg, el_src, el_dst)
    return ua, ua_t, ga, ga_t, lk, (rcp_u, rcp_g)


# ------------------------------------------------------- device program


def build(cfg, ua, ga, lk):
    import os as _os
    KO = _os.environ.get("GNN_KO", "").split(",")
    import concourse.mybir as mybir
    import concourse.tile as tile
    from concourse.tile import add_dep_helper
    from concourse import bacc
    from concourse.bass import BassGpSimd

    dt16 = mybir.dt.float16
    dt32 = mybir.dt.float32
    dti = mybir.dt.int16
    alu = mybir.AluOpType
    act = mybir.ActivationFunctionType
    N = cfg.ncores

    NQ = 4                       # SWDGE queues: gathers on different queues
    nc = bacc.Bacc(None, target_bir_lowering=True, num_devices=N,
                   num_swdge_queues=NQ)

    def inp(name, shape, dt):
        return nc.dram_tensor(name, shape, dt, kind="ExternalInput")

    xu = inp("xu", [cfg.nup, P], dt16)            # full x_user (replicated)
    xg = inp("xg", [cfg.ngp, P], dt16)            # full x_game (replicated)
    xu_s = inp("xu_s", [cfg.tu * cfg.td, P], dt16)     # this core's user shard
    xg_s = inp("xg_s", [cfg.tg * cfg.td, P], dt16)     # this core's game shard
    iota = inp("iota", [P, P], dt16)
    ident = inp("ident", [P, P], dt16)
    ws = {k: inp(k, [P, P], dt16)
          for k in ("wl1u", "wr1u", "wl1g", "wr1g",
                    "wl2u", "wr2u", "wl2g", "wr2g")}
    bs = {k: inp(k, [P, 1], dt32) for k in ("b1u", "b1g", "b2u", "b2g")}
    ua_idx = inp("ua_idx", [P, ua.slots // 16], dti)
    ua_dl = inp("ua_dl", [P, ua.chunks], dt16)
    ga_idx = inp("ga_idx", [P, ga.slots // 16], dti)
    ga_dl = inp("ga_dl", [P, ga.chunks], dt16)
    rcpu = inp("rcpu", [P, cfg.tu * cfg.td], dt16)
    rcpg = inp("rcpg", [P, cfg.tg * cfg.td], dt16)
    lk_ui = inp("lk_ui", [P, lk.slots // 16], dti)
    lk_gi = inp("lk_gi", [P, lk.slots // 16], dti)
    out = nc.dram_tensor("out", [P, lk.chunks], dt32, kind="ExternalOutput")
    dbg = nc.dram_tensor("dbg", [cfg.tu * cfg.td, P], dt16,
                         kind="ExternalOutput") if _os.environ.get("GNN_DBG") \
        else None
    dbgm = nc.dram_tensor("dbgm", [P, cfg.tu * cfg.td], dt16,
                          kind="ExternalOutput") \
        if _os.environ.get("GNN_DBG") == "mean" else None

    u_sh = nc.dram_tensor("u_sh", [cfg.tu * cfg.td, P], dt16)
    g_sh = nc.dram_tensor("g_sh", [cfg.tg * cfg.td, P], dt16)
    u2_sh = nc.dram_tensor("u2_sh", [cfg.tu * cfg.td, P], dt16)
    g2_sh = nc.dram_tensor("g2_sh", [cfg.gs, P], dt16)
    u_full = nc.dram_tensor("u_full", [N * cfg.us, P], dt16,
                            addr_space="Shared")
    g_full = nc.dram_tensor("g_full", [N * cfg.gs, P], dt16,
                            addr_space="Shared")
    g2_full = nc.dram_tensor("g2_full", [N * cfg.gs, P], dt16,
                             addr_space="Shared")
    rg = [list(range(N))]
    gchain = {q: [None, None] for q in range(NQ)}
    qrr = [0]

    def next_q():
        q = qrr[0]
        qrr[0] = (q + 1) % NQ
        return q

    def throttle(gi_inst, q):
        # dma_gather ucode doesn't check SWDGE ring space: keep <=2 gathers
        # (<=2048 descriptors) outstanding per queue by chaining to the
        # queue's 2nd-previous.
        ch = gchain[q]
        if ch[0] is not None:
            add_dep_helper(gi_inst.ins, ch[0],
                           reason="swdge ring throttle")
        ch[0], ch[1] = ch[1], gi_inst.ins

    def allgather(in_ap, out_ap):
        # Run the collective on the (otherwise idle) Scalar queue so its
        # network time overlaps the Pool gather stream instead of blocking
        # it; consumers wait on the cc's completion via tile deps.
        if "cc" in KO:
            return
        # walrus' BIR verifier pins InstCollectiveCompute to the Pool engine
        nc.gpsimd.collective_compute(
            "AllGather", alu.bypass, replica_groups=rg,
            ins=[in_ap], outs=[out_ap])

    with tile.TileContext(nc) as tc:
        with (
            tc.tile_pool(name="res", bufs=1) as res,
            tc.tile_pool(name="gst", bufs=20) as gst,
            tc.tile_pool(name="osb", bufs=20) as osb,
            tc.tile_pool(name="flp", bufs=5) as flp,
            tc.tile_pool(name="psA", bufs=4, space="PSUM") as psA,
            tc.tile_pool(name="psB", bufs=2, space="PSUM") as psB,
            tc.tile_pool(name="lkp", bufs=4) as lkp,
        ):
            def resident(dram, shape, dt, tag):
                t = res.tile(shape, dt, tag=tag)
                nc.sync.dma_start(t[:], dram[:])
                return t

            iota_sb = res.tile([P, 1, P], dt16, tag="iota", name="iota_sb")
            nc.sync.dma_start(iota_sb[:, 0, :], iota[:])
            id_sb = resident(ident, [P, P], dt16, "ident")
            w_sb = {k: resident(v, [P, P], dt16, k) for k, v in ws.items()}
            b_sb = {k: resident(v, [P, 1], dt32, k) for k, v in bs.items()}
            uai = resident(ua_idx, [P, ua.slots // 16], dti, "uai")
            uad = resident(ua_dl, [P, ua.chunks], dt16, "uad")
            gai = resident(ga_idx, [P, ga.slots // 16], dti, "gai")
            gad = resident(ga_dl, [P, ga.chunks], dt16, "gad")
            lui = resident(lk_ui, [P, lk.slots // 16], dti, "lui")
            lgi = resident(lk_gi, [P, lk.slots // 16], dti, "lgi")

            def agg_layer(o, idx_sb, dl_sb, rcp_dram, src_dram, src_rows,
                          xsrc_dram, n_tiles, wl, wr, b, relu,
                          out_dram, out_rows):
                def flush(g, pt):
                    r0 = g * 512
                    w = min(512, n_tiles * cfg.td - r0)
                    rt = flp.tile([P, 512], dt16, tag="rcp")
                    nc.scalar.dma_start(out=rt[:, :w],
                                        in_=rcp_dram[:, r0: r0 + w])
                    mean_sb = flp.tile([P, 512], dt16, tag="mean")
                    if "mm" in KO:
                        nc.vector.memset(mean_sb[:, :w], 0.0)
                    else:
                        nc.vector.tensor_tensor(
                            out=mean_sb[:, :w], in0=pt[:, :w],
                            in1=rt[:, :w], op=alu.mult)
                    if dbgm is not None and out_dram is u_sh:
                        nc.sync.dma_start(dbgm[:, r0: r0 + w],
                                          mean_sb[:, :w])
                    xt = flp.tile([P, 512], dt16, tag="xt")
                    if "xt" in KO:
                        nc.vector.memset(xt[:, :w], 0.0)
                    else:
                        nc.sync.dma_start(
                            out=xt[:, :w],
                            in_=xsrc_dram[r0: r0 + w, :],
                            transpose=True,
                        )
                    ps2 = psB.tile([P, 512], dt32, tag="psB")
                    nc.tensor.matmul(ps2[:, :w], lhsT=wl[:],
                                     rhs=mean_sb[:, :w],
                                     start=True, stop=False)
                    nc.tensor.matmul(ps2[:, :w], lhsT=wr[:], rhs=xt[:, :w],
                                     start=False, stop=True)
                    # bias+relu on DVE (keeps the Scalar queue free for the
                    # collectives, which must not stall behind flush work)
                    ot = flp.tile([P, 512], dt16, tag="ot")
                    nc.vector.tensor_tensor(
                        out=ot[:, :w], in0=ps2[:, :w],
                        in1=b[:].to_broadcast([P, w]), op=alu.add)
                    if relu:
                        nc.vector.tensor_scalar_max(ot[:, :w], ot[:, :w],
                                                    0.0)
                    ps3 = psB.tile([P, 512], dt16, tag="psB3")
                    ut = flp.tile([P, 512], dt16, tag="ut")
                    if "tr" in KO:
                        nc.vector.tensor_copy(ut[:, :w], ot[:, :w])
                    else:
                        for k in range(w // P):
                            nc.tensor.transpose(
                                ps3[:, k * P:(k + 1) * P],
                                ot[:, k * P:(k + 1) * P], id_sb[:])
                        nc.vector.tensor_copy(ut[:, :w], ps3[:, :w])
                    if r0 + w <= out_rows and w % P == 0:
                        nc.sync.dma_start(
                            out=out_dram[r0: r0 + w, :].rearrange(
                                "(k p) h -> p k h", p=P),
                            in_=ut[:, :w].rearrange("p (k h) -> p k h", h=P),
                        )
                    else:
                        for k in range(w // P):
                            rk = r0 + k * P
                            rw = min(P, out_rows - rk)
                            if rw <= 0:
                                break
                            nc.sync.dma_start(out=out_dram[rk: rk + rw, :],
                                              in_=ut[:rw, k * P:(k + 1) * P])

                td = cfg.td
                for g, gi in enumerate(o.group_instrs):
                    stages = {}
                    ohws = {}
                    for i in gi:
                        bnk, soff, n = o.instrs[i]
                        stage = gst.tile([P, cfg.maxi // P, P], dt16,
                                         tag="gst")
                        roff = bnk * BANK
                        rows = min(BANK, src_rows - roff)
                        if "gat" in KO:
                            nc.vector.memset(stage[:, : n // P, :], 0.0)
                        else:
                            q = next_q()
                            throttle(nc.gpsimd.dma_gather(
                                out_ap=stage[:, : n // P, :],
                                in_ap=src_dram[roff: roff + rows, :],
                                idxs_ap=idx_sb[:, soff // 16:
                                               (soff + n) // 16],
                                num_idxs=n,
                                num_idxs_reg=n,
                                elem_size=P,
                                queue_num=q,
                            ), q)
                        stages[i] = stage
                        if "mm" in KO:
                            continue
                        nk = n // P
                        c0 = soff // P
                        ohw = osb.tile([P, cfg.maxi // P, td], dt16,
                                       tag="ohw")
                        nc.vector.tensor_tensor(
                            out=ohw[:, :nk, :],
                            in0=iota_sb[:, :1, :td].to_broadcast(
                                [P, nk, td]),
                            in1=dl_sb[:, c0: c0 + nk].to_broadcast(
                                [P, nk, td]),
                            op=alu.is_equal,
                        )
                        ohws[i] = ohw
                    pt = psA.tile([P, 512], dt32, tag="psA", name="psA_t")
                    if "mm" not in KO:
                        for (c, first, last) in o.group_chunks[g]:
                            i, cl = (int(o.chunk_instr[c, 0]),
                                     int(o.chunk_instr[c, 1]))
                            t = int(o.chunk_tile[c])
                            col = (t % cfg.tpb) * td
                            nc.tensor.matmul(
                                pt[:, col: col + td],
                                lhsT=stages[i][:, cl, :],
                                rhs=ohws[i][:, cl, :],
                                start=bool(first), stop=bool(last),
                            )
                    flush(g, pt)

            # zero the pad rows that L2 transpose-loads will read
            zt = res.tile([P, P], dt16, tag="zero")
            nc.gpsimd.memset(zt[:], 0.0)
            if cfg.tu * cfg.td > cfg.us:
                nc.sync.dma_start(out=u_sh[cfg.us: cfg.tu * cfg.td, :],
                                  in_=zt[: cfg.tu * cfg.td - cfg.us, :])
            if cfg.tg * cfg.td > cfg.gs:
                nc.sync.dma_start(out=g_sh[cfg.gs: cfg.tg * cfg.td, :],
                                  in_=zt[: cfg.tg * cfg.td - cfg.gs, :])

            # ---- layer 1
            agg_layer(ua, uai, uad, rcpu, xg, cfg.ngp, xu_s, cfg.tu,
                      w_sb["wl1u"], w_sb["wr1u"], b_sb["b1u"], True,
                      u_sh, cfg.us)
            allgather(u_sh[: cfg.us, :].opt(), u_full[:].opt())
            agg_layer(ga, gai, gad, rcpg, xu, cfg.nup, xg_s, cfg.tg,
                      w_sb["wl1g"], w_sb["wr1g"], b_sb["b1g"], True,
                      g_sh, cfg.gs)
            allgather(g_sh[: cfg.gs, :].opt(), g_full[:].opt())

            # ---- layer 2 (game side first: it only needs AG_u)
            agg_layer(ga, gai, gad, rcpg, u_full, N * cfg.us, g_sh, cfg.tg,
                      w_sb["wl2g"], w_sb["wr2g"], b_sb["b2g"], False,
                      g2_sh, cfg.gs)
            allgather(g2_sh[:].opt(), g2_full[:].opt())
            agg_layer(ua, uai, uad, rcpu, g_full, N * cfg.gs, u_sh, cfg.tu,
                      w_sb["wl2u"], w_sb["wr2u"], b_sb["b2u"], False,
                      u2_sh, cfg.us)

            # ---- link classifier: interleave u/g gathers with consumption
            ost = res.tile([P, lk.chunks], dt32, tag="ost")
            if "lkg" in KO:
                nc.vector.memset(ost[:], 0.0)
            iu = ig = 0
            cur = {}

            def maybe_gather(side, instrs, i, idxsb, src, rows):
                if i >= len(instrs):
                    return i
                bnk, soff, n = instrs[i]
                stage = lkp.tile([P, cfg.maxi_lk // P, P], dt16,
                                 tag="lk" + side)
                roff = bnk * BANK
                rw = min(BANK, rows - roff)
                if "lgo" in KO:
                    nc.vector.memset(stage[:, : n // P, :], 0.0)
                else:
                    q = next_q()
                    throttle(nc.gpsimd.dma_gather(
                        out_ap=stage[:, : n // P, :],
                        in_ap=src[roff: roff + rw, :],
                        idxs_ap=idxsb[:, soff // 16: (soff + n) // 16],
                        num_idxs=n, num_idxs_reg=n, elem_size=P,
                        queue_num=q,
                    ), q)
                cur[side] = (stage, soff // P, soff // P + n // P)
                return i + 1

            for c in range(lk.chunks if "lkg" not in KO else 0):
                if "u" not in cur or c >= cur["u"][2]:
                    iu = maybe_gather("u", lk.instrs_u, iu, lui, u2_sh,
                                      cfg.tu * cfg.td)
                if "g" not in cur or c >= cur["g"][2]:
                    ig = maybe_gather("g", lk.instrs_g, ig, lgi, g2_full,
                                      N * cfg.gs)
                sut, u0, _ = cur["u"]
                sgt, g0, _ = cur["g"]
                scr = lkp.tile([P, P], dt32, tag="lscr")
                nc.vector.tensor_tensor(
                    out=scr[:],
                    in0=sut[:, c - u0, :],
                    in1=sgt[:, c - g0, :],
                    op=alu.mult,
                )
                nc.vector.tensor_reduce(
                    out=ost[:, c: c + 1],
                    in_=scr[:],
                    axis=mybir.AxisListType.X,
                    op=alu.add,
                )
            nc.sync.dma_start(out=out[:], in_=ost[:])
            if dbg is not None:
                dt = res.tile([P, 512], dt16, tag="dbgt")
                for r0 in range(0, cfg.tu * cfg.td, 512):
                    w = min(512, cfg.tu * cfg.td - r0)
                    src = {"u": u_sh, "g": g_sh, "u2": u2_sh,
                           "mean": u_sh}[_os.environ["GNN_DBG"]]
                    nc.sync.dma_start(dt[:, :w].rearrange(
                        "p (k h) -> p k h", h=P)[:, : w // P, :],
                        src[r0: r0 + w, :].rearrange(
                            "(k p) h -> p k h", p=P))
                    nc.sync.dma_start(
                        dbg[r0: r0 + w, :].rearrange("(k p) h -> p k h", p=P),
                        dt[:, :w].rearrange("p (k h) -> p k h", h=P)[
                            :, : w // P, :])


    nc.compile()
    return nc


# ------------------------------------------------------------- kernel()


def _make_inputs(cfg, xu_np, xg_np, wmap, bmap, ua_t, ga_t, lk, rcp):
    iota_np = np.tile(np.arange(P, dtype=f16), (P, 1))
    id_np = np.eye(P, dtype=f16)
    ua_i, ua_d = ua_t
    ga_i, ga_d = ga_t
    rcp_u, rcp_g = rcp
    in_maps = []
    for c in range(cfg.ncores):
        m = {"xu": xu_np, "xg": xg_np, "iota": iota_np, "ident": id_np,
             "xu_s": xu_np[c * cfg.us: c * cfg.us + cfg.tu * cfg.td],
             "xg_s": xg_np[c * cfg.gs: c * cfg.gs + cfg.tg * cfg.td]}
        for k, v in wmap.items():
            m[k] = np.asarray(v, f32).astype(f16)
        for k, v in bmap.items():
            m[k] = np.asarray(v, f32).reshape(P, 1)
        m["ua_idx"] = _pack_idx(ua_i[c])
        m["ua_dl"] = _pack_cols(ua_d[c])
        m["ga_idx"] = _pack_idx(ga_i[c])
        m["ga_dl"] = _pack_cols(ga_d[c])
        m["rcpu"] = np.ascontiguousarray(np.tile(rcp_u[c], (P, 1)))
        m["rcpg"] = np.ascontiguousarray(np.tile(rcp_g[c], (P, 1)))
        m["lk_ui"] = _pack_idx(lk.uidx[c])
        m["lk_gi"] = _pack_idx(lk.gidx[c])
        in_maps.append(m)
    return in_maps


_cache = {}


def run(cfg, inputs_np, run_kwargs=None):
    """Full pipeline given reference-named inputs; returns [EL] fp32."""
    from concourse.bass_utils import run_bass_kernel_spmd

    ii = inputs_np
    edge_src = np.asarray(ii["edge_src"], np.int64)
    edge_dst = np.asarray(ii["edge_dst"], np.int64)
    el_src = np.asarray(ii["el_src"], np.int64)
    el_dst = np.asarray(ii["el_dst"], np.int64)

    xu_np = np.zeros((cfg.nup, P), f16)
    xu_np[: cfg.nu] = np.asarray(ii["user_emb"], f32)[
        np.asarray(ii["user_node_id"])]
    xg_np = np.zeros((cfg.ngp, P), f16)
    xg_np[: cfg.ng] = (
        np.asarray(ii["game_x"], f32) @ np.asarray(ii["game_lin_w"], f32)
        + np.asarray(ii["game_lin_b"], f32)
        + np.asarray(ii["game_emb"], f32)[np.asarray(ii["game_node_id"])])

    key = (edge_src[:64].tobytes(), edge_dst[:64].tobytes(),
           int(edge_src.sum()), int(edge_dst.sum()))
    if key in _cache:
        ua, ua_t, ga, ga_t, lk, rcp, nc = _cache[key]
    else:
        ua, ua_t, ga, ga_t, lk, rcp = prep(cfg, edge_src, edge_dst,
                                           el_src, el_dst)
        nc = build(cfg, ua, ga, lk)
        _cache[key] = (ua, ua_t, ga, ga_t, lk, rcp, nc)

    wmap = {"wl1u": ii["w1l_r"], "wr1u": ii["w1r_r"],
            "wl1g": ii["w1l_o"], "wr1g": ii["w1r_o"],
            "wl2u": ii["w2l_r"], "wr2u": ii["w2r_r"],
            "wl2g": ii["w2l_o"], "wr2g": ii["w2r_o"]}
    bmap = {"b1u": ii["b1l_r"], "b1g": ii["b1l_o"],
            "b2u": ii["b2l_r"], "b2g": ii["b2l_o"]}
    in_maps = _make_inputs(cfg, xu_np, xg_np, wmap, bmap, ua_t, ga_t, lk,
                           rcp)

    res = run_bass_kernel_spmd(nc, in_maps,
                               core_ids=list(range(cfg.ncores)),
                               **(run_kwargs or {}))
    global LAST_RESULTS
    LAST_RESULTS = res

    result = np.zeros(cfg.el, f32)
    for c in range(cfg.ncores):
        oc = np.asarray(res.results[c]["out"], f32)   # [128, chunks]
        flat = oc.T.reshape(-1)                        # slot j = flat[j]
        valid = lk.orig[c] >= 0
        result[lk.orig[c][valid]] = flat[valid]
    return result


def kernel(user_node_id, game_node_id, game_x, edge_src, edge_dst,
           el_src, el_dst, user_emb, game_emb, game_lin_w, game_lin_b,
           w1l_o, b1l_o, w1r_o, w1l_r, b1l_r, w1r_r,
           w2l_o, b2l_o, w2r_o, w2l_r, b2l_r, w2r_r):
    cfg = CFG(8, 100000, 50000, 128, 1000000, 500000)
    return run(cfg, dict(
        user_node_id=user_node_id, game_node_id=game_node_id,
        game_x=game_x, edge_src=edge_src, edge_dst=edge_dst,
        el_src=el_src, el_dst=el_dst, user_emb=user_emb,
        game_emb=game_emb, game_lin_w=game_lin_w, game_lin_b=game_lin_b,
        w1l_o=w1l_o, b1l_o=b1l_o, w1r_o=w1r_o,
        w1l_r=w1l_r, b1l_r=b1l_r, w1r_r=w1r_r,
        w2l_o=w2l_o, b2l_o=b2l_o, w2r_o=w2r_o,
        w2l_r=w2l_r, b2l_r=b2l_r, w2r_r=w2r_r))



# revision 28
# speedup vs baseline: 1.2540x; 1.0593x over previous
"""nn_GNNModel: 2-layer bipartite GraphSAGE + link classifier on 8 TRN2 cores.

Edge-parallel sharding by destination node: core c owns users [c*12500,
(c+1)*12500) and games [c*6250, (c+1)*6250). Each aggregation gathers
source-node feature rows (fp16, 256B) with gpsimd.dma_gather and reduces
them into PSUM via a scaled one-hot matmul (the mean's 1/deg folded into
the one-hot values). The dense SAGE transform runs at PSUM flush; outputs
are transposed back to row-major and AllGather-ed so the next layer can
gather from the full table. The link classifier is a pair of gathers plus
a fused multiply-reduce per 128 label edges.
"""

import numpy as np

P = 128
BANK = 32768
f16 = np.float16
f32 = np.float32


def _cdiv(a, b):
    return -(-a // b)


# ---------------------------------------------------------------- config


class CFG:
    def __init__(self, ncores, nu, ng, h, e, el,
                 maxi=1024, maxi_lk=1024, grp_u=2, grp_g=1, td=64):
        assert h == P
        self.ncores, self.nu, self.ng, self.h = ncores, nu, ng, h
        self.e, self.el = e, el
        self.td = td                                   # dst-tile width
        self.tpb = 512 // td                           # dst tiles per psum bank
        self.us, self.gs = nu // ncores, ng // ncores  # per-core shards
        self.tu = _cdiv(self.us, td)                   # user tiles / core
        self.tg = _cdiv(self.gs, td)                   # game tiles / core
        self.nup = ncores * self.tu * td               # padded user rows
        self.ngp = ncores * self.tg * td               # padded game rows
        self.maxi, self.maxi_lk = maxi, maxi_lk
        self.grp_u = grp_u * self.tpb              # tiles per gather group
        self.grp_g = grp_g * self.tpb


# ------------------------------------------------------- host-side prep


class Orient:
    """Core-independent slot layout for one aggregation orientation.

    Slots are grouped (group of `grp` dst tiles) -> (source bank) ->
    (dst tile), each (tile, bank) run padded to a multiple of 128 and
    sized to the max count over cores so a single NEFF fits all cores.
    """

    def __init__(self, cfg, dst, src, n_src_pad, shard, n_tiles, grp):
        td = cfg.td
        nb = _cdiv(n_src_pad, BANK)
        core = dst // shard
        tile = (dst - core * shard) // td
        bank = src >> 15
        key = (core * n_tiles + tile) * nb + bank
        counts = np.bincount(key, minlength=cfg.ncores * n_tiles * nb)
        counts = counts.reshape(cfg.ncores, n_tiles, nb)
        cap = _cdiv(counts.max(axis=0), P) * P         # [n_tiles, nb]
        cap[cap.sum(axis=1) == 0, 0] = P               # tile >=1 chunk
        self.nb, self.n_tiles, self.grp = nb, n_tiles, grp

        grp = cfg.tpb                                  # tiles per psum bank
        self.grp = grp
        rank = np.full((n_tiles, nb), -1, np.int64)
        run_off, run_cap = [], []
        off = 0
        for g in range(_cdiv(n_tiles, grp)):
            ts = range(g * grp, min((g + 1) * grp, n_tiles))
            for b in range(nb):
                for t in ts:
                    if cap[t, b]:
                        rank[t, b] = len(run_off)
                        run_off.append(off)
                        run_cap.append(int(cap[t, b]))
                        off += int(cap[t, b])
        self.rank = rank
        self.run_off = np.asarray(run_off, np.int64)
        self.slots = off
        self.chunks = off // P

        ct = np.empty(self.chunks, np.int64)
        for (t, b), idx in np.ndenumerate(rank):
            if idx >= 0:
                o = int(self.run_off[idx])
                ct[o // P:(o + int(cap[t, b])) // P] = t
        self.chunk_tile = ct
        self.tpb = cfg.tpb

        # gather instructions: (bank, slot_off, n), n <= maxi, 128-aligned;
        # grouped so all instrs of psum-group g are consecutive
        self.instrs = []
        self.group_instrs = []      # per group: list of instr indices
        self.group_tiles = []       # per group: list of tile ids
        for g in range(_cdiv(n_tiles, grp)):
            ts = range(g * grp, min((g + 1) * grp, n_tiles))
            self.group_tiles.append(list(ts))
            gi = []
            for b in range(nb):
                runs = [t for t in ts if cap[t, b]]
                if not runs:
                    continue
                start = int(self.run_off[rank[runs[0], b]])
                total = int(sum(cap[t, b] for t in runs))
                while total > 0:
                    take = min(total, cfg.maxi)
                    gi.append(len(self.instrs))
                    self.instrs.append((b, start, take))
                    start += take
                    total -= take
            self.group_instrs.append(gi)
        # chunk -> (instr idx, chunk-within-instr)
        self.chunk_instr = np.zeros((self.chunks, 2), np.int64)
        for i, (b, soff, n) in enumerate(self.instrs):
            for cl in range(n // P):
                self.chunk_instr[soff // P + cl] = (i, cl)
        # per group: emission order of chunks, tile-major then bank
        self.group_chunks = []      # per group: [(chunk, start, stop)]
        for g, ts in enumerate(self.group_tiles):
            order = []
            for t in ts:
                tcs = []
                for b in range(nb):
                    if rank[t, b] >= 0:
                        o = int(self.run_off[rank[t, b]])
                        tcs.extend(range(o // P, (o + int(cap[t, b])) // P))
                for j, c in enumerate(tcs):
                    order.append((c, j == 0, j == len(tcs) - 1))
            self.group_chunks.append(order)


def _fill_orient(cfg, o, dst, src, shard):
    """Per-core slot tables: int16 gather idx, fp16 dstloc.

    Pad slots keep dstloc=255, which never matches the iota (td<=128), so
    they drop out of the one-hot without a separate scale table. The mean's
    1/deg is applied per-dst at flush time instead.
    """
    core = dst // shard
    loc = dst - core * shard
    tile = loc // cfg.td
    dstloc = loc - tile * cfg.td
    lk = o.rank[tile, src >> 15]
    assert (lk >= 0).all()
    idx16 = np.zeros((cfg.ncores, o.slots), np.int16)
    dl = np.full((cfg.ncores, o.slots), 255, f16)
    for c in range(cfg.ncores):
        m = core == c
        lkc = lk[m]
        # secondary sort by src row: HBM row-buffer locality in the gathers
        ordr = np.lexsort((src[m], lkc))
        lks = lkc[ordr]
        cnts = np.bincount(lks, minlength=len(o.run_off))
        starts = np.concatenate([[0], np.cumsum(cnts)[:-1]])
        within = np.arange(len(lks)) - starts[lks]
        slots = o.run_off[lks] + within
        idx16[c, slots] = (src[m][ordr] & (BANK - 1)).astype(np.int16)
        dl[c, slots] = dstloc[m][ordr].astype(f16)
    return idx16, dl


def _pack_idx(a):
    """[SLOTS] int16 -> [128, SLOTS//16]: slot j -> [j%16, j//16], x8."""
    n = a.shape[-1]
    w = np.swapaxes(a.reshape(n // 16, 16), -1, -2)
    return np.ascontiguousarray(np.tile(w, (8, 1)))


def _pack_cols(a):
    """[SLOTS] -> [128, SLOTS//128]: slot j -> [j%128, j//128]."""
    n = a.shape[-1]
    return np.ascontiguousarray(np.swapaxes(a.reshape(n // P, P), -1, -2))


class LinkPrep:
    def __init__(self, cfg, el_src, el_dst):
        nbg = _cdiv(cfg.ngp, BANK)
        core = el_src // cfg.us
        bank = el_dst >> 15
        counts = np.bincount(core * nbg + bank, minlength=cfg.ncores * nbg)
        capl = _cdiv(counts.reshape(cfg.ncores, nbg).max(axis=0), P) * P
        np.maximum(capl, P, out=capl)
        self.nbg = nbg
        self.off = np.concatenate([[0], np.cumsum(capl)])
        self.slots = int(self.off[-1])
        self.chunks = self.slots // P
        self.instrs_g = []
        for b in range(nbg):
            start, total = int(self.off[b]), int(capl[b])
            while total > 0:
                take = min(total, cfg.maxi_lk)
                self.instrs_g.append((b, start, take))
                start += take
                total -= take
        self.instrs_u = []
        start, total = 0, self.slots
        while total > 0:
            take = min(total, cfg.maxi_lk)
            self.instrs_u.append((0, start, take))
            start += take
            total -= take

        self.uidx = np.zeros((cfg.ncores, self.slots), np.int16)
        self.gidx = np.zeros((cfg.ncores, self.slots), np.int16)
        self.orig = np.full((cfg.ncores, self.slots), -1, np.int64)
        for c in range(cfg.ncores):
            m = core == c
            es, ed = el_src[m], el_dst[m]
            oi = np.nonzero(m)[0]
            b = ed >> 15
            ordr = np.argsort(b, kind="stable")
            bs_ = b[ordr]
            cnts = np.bincount(bs_, minlength=nbg)
            starts = np.concatenate([[0], np.cumsum(cnts)[:-1]])
            within = np.arange(len(bs_)) - starts[bs_]
            slots = self.off[bs_] + within
            self.uidx[c, slots] = (es[ordr] - c * cfg.us).astype(np.int16)
            self.gidx[c, slots] = (ed[ordr] & (BANK - 1)).astype(np.int16)
            self.orig[c, slots] = oi[ordr]


def prep(cfg, edge_src, edge_dst, el_src, el_dst):
    su = 1.0 / np.maximum(
        np.bincount(edge_src, minlength=cfg.nu).astype(f32), 1.0)
    sg = 1.0 / np.maximum(
        np.bincount(edge_dst, minlength=cfg.ng).astype(f32), 1.0)
    # per-core, per-local-dst recip tables, padded rows -> 1.0
    rcp_u = np.ones((cfg.ncores, cfg.tu * cfg.td), f16)
    rcp_g = np.ones((cfg.ncores, cfg.tg * cfg.td), f16)
    for c in range(cfg.ncores):
        rcp_u[c, : cfg.us] = su[c * cfg.us: (c + 1) * cfg.us]
        rcp_g[c, : cfg.gs] = sg[c * cfg.gs: (c + 1) * cfg.gs]
    # user-agg: dst=edge_src (users), src=edge_dst (games)
    ua = Orient(cfg, edge_src, edge_dst, cfg.ngp, cfg.us, cfg.tu, cfg.grp_u)
    ua_t = _fill_orient(cfg, ua, edge_src, edge_dst, cfg.us)
    # game-agg: dst=edge_dst (games), src=edge_src (users)
    ga = Orient(cfg, edge_dst, edge_src, cfg.nup, cfg.gs, cfg.tg, cfg.grp_g)
    ga_t = _fill_orient(cfg, ga, edge_dst, edge_src, cfg.gs)
    lk = LinkPrep(cfg, el_src, el_dst)
    return ua, ua_t, ga, ga_t, lk, (rcp_u, rcp_g)


# ------------------------------------------------------- device program


def build(cfg, ua, ga, lk):
    import os as _os
    KO = _os.environ.get("GNN_KO", "").split(",")
    import concourse.mybir as mybir
    import concourse.tile as tile
    from concourse.tile import add_dep_helper
    from concourse import bacc
    from concourse.bass import BassGpSimd

    dt16 = mybir.dt.float16
    dt32 = mybir.dt.float32
    dti = mybir.dt.int16
    alu = mybir.AluOpType
    act = mybir.ActivationFunctionType
    N = cfg.ncores

    NQ = 4                       # SWDGE queues: gathers on different queues
    nc = bacc.Bacc(None, target_bir_lowering=True, num_devices=N,
                   num_swdge_queues=NQ)

    def inp(name, shape, dt):
        return nc.dram_tensor(name, shape, dt, kind="ExternalInput")

    xu = inp("xu", [cfg.nup, P], dt16)            # full x_user (replicated)
    xg = inp("xg", [cfg.ngp, P], dt16)            # full x_game (replicated)
    xu_s = inp("xu_s", [cfg.tu * cfg.td, P], dt16)     # this core's user shard
    xg_s = inp("xg_s", [cfg.tg * cfg.td, P], dt16)     # this core's game shard
    iota = inp("iota", [P, P], dt16)
    ident = inp("ident", [P, P], dt16)
    ws = {k: inp(k, [P, P], dt16)
          for k in ("wl1u", "wr1u", "wl1g", "wr1g",
                    "wl2u", "wr2u", "wl2g", "wr2g")}
    bs = {k: inp(k, [P, 1], dt32) for k in ("b1u", "b1g", "b2u", "b2g")}
    ua_idx = inp("ua_idx", [P, ua.slots // 16], dti)
    ua_dl = inp("ua_dl", [P, ua.chunks], dt16)
    ga_idx = inp("ga_idx", [P, ga.slots // 16], dti)
    ga_dl = inp("ga_dl", [P, ga.chunks], dt16)
    rcpu = inp("rcpu", [P, cfg.tu * cfg.td], dt16)
    rcpg = inp("rcpg", [P, cfg.tg * cfg.td], dt16)
    lk_ui = inp("lk_ui", [P, lk.slots // 16], dti)
    lk_gi = inp("lk_gi", [P, lk.slots // 16], dti)
    out = nc.dram_tensor("out", [P, lk.chunks], dt32, kind="ExternalOutput")
    dbg = nc.dram_tensor("dbg", [cfg.tu * cfg.td, P], dt16,
                         kind="ExternalOutput") if _os.environ.get("GNN_DBG") \
        else None
    dbgm = nc.dram_tensor("dbgm", [P, cfg.tu * cfg.td], dt16,
                          kind="ExternalOutput") \
        if _os.environ.get("GNN_DBG") == "mean" else None

    u_sh = nc.dram_tensor("u_sh", [cfg.tu * cfg.td, P], dt16)
    g_sh = nc.dram_tensor("g_sh", [cfg.tg * cfg.td, P], dt16)
    u2_sh = nc.dram_tensor("u2_sh", [cfg.tu * cfg.td, P], dt16)
    g2_sh = nc.dram_tensor("g2_sh", [cfg.gs, P], dt16)
    u_full = nc.dram_tensor("u_full", [N * cfg.us, P], dt16,
                            addr_space="Shared")
    g_full = nc.dram_tensor("g_full", [N * cfg.gs, P], dt16,
                            addr_space="Shared")
    g2_full = nc.dram_tensor("g2_full", [N * cfg.gs, P], dt16,
                             addr_space="Shared")
    rg = [list(range(N))]
    gchain = {q: [None, None] for q in range(NQ)}
    qrr = [0]

    def next_q():
        q = qrr[0]
        qrr[0] = (q + 1) % NQ
        return q

    def throttle(gi_inst, q):
        # dma_gather ucode doesn't check SWDGE ring space: keep <=2 gathers
        # (<=2048 descriptors) outstanding per queue by chaining to the
        # queue's 2nd-previous.
        ch = gchain[q]
        if ch[0] is not None:
            add_dep_helper(gi_inst.ins, ch[0],
                           reason="swdge ring throttle")
        ch[0], ch[1] = ch[1], gi_inst.ins

    def allgather(in_ap, out_ap):
        # Run the collective on the (otherwise idle) Scalar queue so its
        # network time overlaps the Pool gather stream instead of blocking
        # it; consumers wait on the cc's completion via tile deps.
        if "cc" in KO:
            return
        # walrus' BIR verifier pins InstCollectiveCompute to the Pool engine
        nc.gpsimd.collective_compute(
            "AllGather", alu.bypass, replica_groups=rg,
            ins=[in_ap], outs=[out_ap])

    with tile.TileContext(nc) as tc:
        with (
            tc.tile_pool(name="res", bufs=1) as res,
            tc.tile_pool(name="gst", bufs=20) as gst,
            tc.tile_pool(name="osb", bufs=20) as osb,
            tc.tile_pool(name="flp", bufs=5) as flp,
            tc.tile_pool(name="psA", bufs=4, space="PSUM") as psA,
            tc.tile_pool(name="psB", bufs=2, space="PSUM") as psB,
            tc.tile_pool(name="lkp", bufs=4) as lkp,
        ):
            def resident(dram, shape, dt, tag):
                t = res.tile(shape, dt, tag=tag)
                nc.sync.dma_start(t[:], dram[:])
                return t

            iota_sb = res.tile([P, 1, P], dt16, tag="iota", name="iota_sb")
            nc.sync.dma_start(iota_sb[:, 0, :], iota[:])
            id_sb = resident(ident, [P, P], dt16, "ident")
            w_sb = {k: resident(v, [P, P], dt16, k) for k, v in ws.items()}
            b_sb = {k: resident(v, [P, 1], dt32, k) for k, v in bs.items()}
            uai = resident(ua_idx, [P, ua.slots // 16], dti, "uai")
            uad = resident(ua_dl, [P, ua.chunks], dt16, "uad")
            gai = resident(ga_idx, [P, ga.slots // 16], dti, "gai")
            gad = resident(ga_dl, [P, ga.chunks], dt16, "gad")
            lui = resident(lk_ui, [P, lk.slots // 16], dti, "lui")
            lgi = resident(lk_gi, [P, lk.slots // 16], dti, "lgi")

            def agg_layer(o, idx_sb, dl_sb, rcp_dram, src_dram, src_rows,
                          xsrc_dram, n_tiles, wl, wr, b, relu,
                          out_dram, out_rows):
                def flush(g, pt):
                    r0 = g * 512
                    w = min(512, n_tiles * cfg.td - r0)
                    rt = flp.tile([P, 512], dt16, tag="rcp")
                    nc.scalar.dma_start(out=rt[:, :w],
                                        in_=rcp_dram[:, r0: r0 + w])
                    mean_sb = flp.tile([P, 512], dt16, tag="mean")
                    if "mm" in KO:
                        nc.vector.memset(mean_sb[:, :w], 0.0)
                    else:
                        nc.vector.tensor_tensor(
                            out=mean_sb[:, :w], in0=pt[:, :w],
                            in1=rt[:, :w], op=alu.mult)
                    if dbgm is not None and out_dram is u_sh:
                        nc.sync.dma_start(dbgm[:, r0: r0 + w],
                                          mean_sb[:, :w])
                    xt = flp.tile([P, 512], dt16, tag="xt")
                    if "xt" in KO:
                        nc.vector.memset(xt[:, :w], 0.0)
                    else:
                        nc.sync.dma_start(
                            out=xt[:, :w],
                            in_=xsrc_dram[r0: r0 + w, :],
                            transpose=True,
                        )
                    ps2 = psB.tile([P, 512], dt32, tag="psB")
                    nc.tensor.matmul(ps2[:, :w], lhsT=wl[:],
                                     rhs=mean_sb[:, :w],
                                     start=True, stop=False)
                    nc.tensor.matmul(ps2[:, :w], lhsT=wr[:], rhs=xt[:, :w],
                                     start=False, stop=True)
                    # bias+relu on DVE (keeps the Scalar queue free for the
                    # collectives, which must not stall behind flush work)
                    ot = flp.tile([P, 512], dt16, tag="ot")
                    nc.vector.tensor_tensor(
                        out=ot[:, :w], in0=ps2[:, :w],
                        in1=b[:].to_broadcast([P, w]), op=alu.add)
                    if relu:
                        nc.vector.tensor_scalar_max(ot[:, :w], ot[:, :w],
                                                    0.0)
                    ps3 = psB.tile([P, 512], dt16, tag="psB3")
                    ut = flp.tile([P, 512], dt16, tag="ut")
                    if "tr" in KO:
                        nc.vector.tensor_copy(ut[:, :w], ot[:, :w])
                    else:
                        for k in range(w // P):
                            nc.tensor.transpose(
                                ps3[:, k * P:(k + 1) * P],
                                ot[:, k * P:(k + 1) * P], id_sb[:])
                        nc.vector.tensor_copy(ut[:, :w], ps3[:, :w])
                    if r0 + w <= out_rows and w % P == 0:
                        nc.sync.dma_start(
                            out=out_dram[r0: r0 + w, :].rearrange(
                                "(k p) h -> p k h", p=P),
                            in_=ut[:, :w].rearrange("p (k h) -> p k h", h=P),
                        )
                    else:
                        for k in range(w // P):
                            rk = r0 + k * P
                            rw = min(P, out_rows - rk)
                            if rw <= 0:
                                break
                            nc.sync.dma_start(out=out_dram[rk: rk + rw, :],
                                              in_=ut[:rw, k * P:(k + 1) * P])

                td = cfg.td

                def group_thunk(g):
                    def run():
                        gi = o.group_instrs[g]
                        stages = {}
                        ohws = {}
                        for i in gi:
                            bnk, soff, n = o.instrs[i]
                            stage = gst.tile([P, cfg.maxi // P, P], dt16,
                                             tag="gst")
                            roff = bnk * BANK
                            rows = min(BANK, src_rows - roff)
                            if "gat" in KO:
                                nc.vector.memset(stage[:, : n // P, :], 0.0)
                            else:
                                q = next_q()
                                throttle(nc.gpsimd.dma_gather(
                                    out_ap=stage[:, : n // P, :],
                                    in_ap=src_dram[roff: roff + rows, :],
                                    idxs_ap=idx_sb[:, soff // 16:
                                                   (soff + n) // 16],
                                    num_idxs=n,
                                    num_idxs_reg=n,
                                    elem_size=P,
                                    queue_num=q,
                                ), q)
                            stages[i] = stage
                            if "mm" in KO:
                                continue
                            nk = n // P
                            c0 = soff // P
                            ohw = osb.tile([P, cfg.maxi // P, td], dt16,
                                           tag="ohw")
                            nc.vector.tensor_tensor(
                                out=ohw[:, :nk, :],
                                in0=iota_sb[:, :1, :td].to_broadcast(
                                    [P, nk, td]),
                                in1=dl_sb[:, c0: c0 + nk].to_broadcast(
                                    [P, nk, td]),
                                op=alu.is_equal,
                            )
                            ohws[i] = ohw
                        pt = psA.tile([P, 512], dt32, tag="psA",
                                      name="psA_t")
                        if "mm" not in KO:
                            for (c, first, last) in o.group_chunks[g]:
                                i, cl = (int(o.chunk_instr[c, 0]),
                                         int(o.chunk_instr[c, 1]))
                                t = int(o.chunk_tile[c])
                                col = (t % cfg.tpb) * td
                                nc.tensor.matmul(
                                    pt[:, col: col + td],
                                    lhsT=stages[i][:, cl, :],
                                    rhs=ohws[i][:, cl, :],
                                    start=bool(first), stop=bool(last),
                                )
                        flush(g, pt)
                    return run

                return [group_thunk(g)
                        for g in range(len(o.group_instrs))]

            def drive(a, b, na, nb):
                # interleave two thunk lists, na from a then nb from b
                ia = ib = 0
                while ia < len(a) or ib < len(b):
                    for _ in range(na):
                        if ia < len(a):
                            a[ia]()
                            ia += 1
                    for _ in range(nb):
                        if ib < len(b):
                            b[ib]()
                            ib += 1

            # zero the pad rows that L2 transpose-loads will read
            zt = res.tile([P, P], dt16, tag="zero")
            nc.gpsimd.memset(zt[:], 0.0)
            if cfg.tu * cfg.td > cfg.us:
                nc.sync.dma_start(out=u_sh[cfg.us: cfg.tu * cfg.td, :],
                                  in_=zt[: cfg.tu * cfg.td - cfg.us, :])
            if cfg.tg * cfg.td > cfg.gs:
                nc.sync.dma_start(out=g_sh[cfg.gs: cfg.tg * cfg.td, :],
                                  in_=zt[: cfg.tg * cfg.td - cfg.gs, :])

            # ---- layer 1: interleave the two aggregations' gather streams
            # so one side's compute hides the other side's HBM latency
            l1u = agg_layer(ua, uai, uad, rcpu, xg, cfg.ngp, xu_s, cfg.tu,
                            w_sb["wl1u"], w_sb["wr1u"], b_sb["b1u"], True,
                            u_sh, cfg.us)
            l1g = agg_layer(ga, gai, gad, rcpg, xu, cfg.nup, xg_s, cfg.tg,
                            w_sb["wl1g"], w_sb["wr1g"], b_sb["b1g"], True,
                            g_sh, cfg.gs)
            drive(l1u, l1g, 2, 1)
            allgather(u_sh[: cfg.us, :].opt(), u_full[:].opt())
            allgather(g_sh[: cfg.gs, :].opt(), g_full[:].opt())

            # ---- layer 2 (game side first-heavy: its output feeds AG_g2)
            l2g = agg_layer(ga, gai, gad, rcpg, u_full, N * cfg.us, g_sh,
                            cfg.tg, w_sb["wl2g"], w_sb["wr2g"], b_sb["b2g"],
                            False, g2_sh, cfg.gs)
            l2u = agg_layer(ua, uai, uad, rcpu, g_full, N * cfg.gs, u_sh,
                            cfg.tu, w_sb["wl2u"], w_sb["wr2u"], b_sb["b2u"],
                            False, u2_sh, cfg.us)
            drive(l2g, l2u, 1, 1)
            allgather(g2_sh[:].opt(), g2_full[:].opt())

            # ---- link classifier: interleave u/g gathers with consumption
            ost = res.tile([P, lk.chunks], dt32, tag="ost")
            if "lkg" in KO:
                nc.vector.memset(ost[:], 0.0)
            iu = ig = 0
            cur = {}

            def maybe_gather(side, instrs, i, idxsb, src, rows):
                if i >= len(instrs):
                    return i
                bnk, soff, n = instrs[i]
                stage = lkp.tile([P, cfg.maxi_lk // P, P], dt16,
                                 tag="lk" + side)
                roff = bnk * BANK
                rw = min(BANK, rows - roff)
                if "lgo" in KO:
                    nc.vector.memset(stage[:, : n // P, :], 0.0)
                else:
                    q = next_q()
                    throttle(nc.gpsimd.dma_gather(
                        out_ap=stage[:, : n // P, :],
                        in_ap=src[roff: roff + rw, :],
                        idxs_ap=idxsb[:, soff // 16: (soff + n) // 16],
                        num_idxs=n, num_idxs_reg=n, elem_size=P,
                        queue_num=q,
                    ), q)
                cur[side] = (stage, soff // P, soff // P + n // P)
                return i + 1

            for c in range(lk.chunks if "lkg" not in KO else 0):
                if "u" not in cur or c >= cur["u"][2]:
                    iu = maybe_gather("u", lk.instrs_u, iu, lui, u2_sh,
                                      cfg.tu * cfg.td)
                if "g" not in cur or c >= cur["g"][2]:
                    ig = maybe_gather("g", lk.instrs_g, ig, lgi, g2_full,
                                      N * cfg.gs)
                sut, u0, _ = cur["u"]
                sgt, g0, _ = cur["g"]
                scr = lkp.tile([P, P], dt32, tag="lscr")
                nc.vector.tensor_tensor(
                    out=scr[:],
                    in0=sut[:, c - u0, :],
                    in1=sgt[:, c - g0, :],
                    op=alu.mult,
                )
                nc.vector.tensor_reduce(
                    out=ost[:, c: c + 1],
                    in_=scr[:],
                    axis=mybir.AxisListType.X,
                    op=alu.add,
                )
            nc.sync.dma_start(out=out[:], in_=ost[:])
            if dbg is not None:
                dt = res.tile([P, 512], dt16, tag="dbgt")
                for r0 in range(0, cfg.tu * cfg.td, 512):
                    w = min(512, cfg.tu * cfg.td - r0)
                    src = {"u": u_sh, "g": g_sh, "u2": u2_sh,
                           "mean": u_sh}[_os.environ["GNN_DBG"]]
                    nc.sync.dma_start(dt[:, :w].rearrange(
                        "p (k h) -> p k h", h=P)[:, : w // P, :],
                        src[r0: r0 + w, :].rearrange(
                            "(k p) h -> p k h", p=P))
                    nc.sync.dma_start(
                        dbg[r0: r0 + w, :].rearrange("(k p) h -> p k h", p=P),
                        dt[:, :w].rearrange("p (k h) -> p k h", h=P)[
                            :, : w // P, :])


    nc.compile()
    return nc


# ------------------------------------------------------------- kernel()


def _make_inputs(cfg, xu_np, xg_np, wmap, bmap, ua_t, ga_t, lk, rcp):
    iota_np = np.tile(np.arange(P, dtype=f16), (P, 1))
    id_np = np.eye(P, dtype=f16)
    ua_i, ua_d = ua_t
    ga_i, ga_d = ga_t
    rcp_u, rcp_g = rcp
    in_maps = []
    for c in range(cfg.ncores):
        m = {"xu": xu_np, "xg": xg_np, "iota": iota_np, "ident": id_np,
             "xu_s": xu_np[c * cfg.us: c * cfg.us + cfg.tu * cfg.td],
             "xg_s": xg_np[c * cfg.gs: c * cfg.gs + cfg.tg * cfg.td]}
        for k, v in wmap.items():
            m[k] = np.asarray(v, f32).astype(f16)
        for k, v in bmap.items():
            m[k] = np.asarray(v, f32).reshape(P, 1)
        m["ua_idx"] = _pack_idx(ua_i[c])
        m["ua_dl"] = _pack_cols(ua_d[c])
        m["ga_idx"] = _pack_idx(ga_i[c])
        m["ga_dl"] = _pack_cols(ga_d[c])
        m["rcpu"] = np.ascontiguousarray(np.tile(rcp_u[c], (P, 1)))
        m["rcpg"] = np.ascontiguousarray(np.tile(rcp_g[c], (P, 1)))
        m["lk_ui"] = _pack_idx(lk.uidx[c])
        m["lk_gi"] = _pack_idx(lk.gidx[c])
        in_maps.append(m)
    return in_maps


_cache = {}


def run(cfg, inputs_np, run_kwargs=None):
    """Full pipeline given reference-named inputs; returns [EL] fp32."""
    from concourse.bass_utils import run_bass_kernel_spmd

    ii = inputs_np
    edge_src = np.asarray(ii["edge_src"], np.int64)
    edge_dst = np.asarray(ii["edge_dst"], np.int64)
    el_src = np.asarray(ii["el_src"], np.int64)
    el_dst = np.asarray(ii["el_dst"], np.int64)

    xu_np = np.zeros((cfg.nup, P), f16)
    xu_np[: cfg.nu] = np.asarray(ii["user_emb"], f32)[
        np.asarray(ii["user_node_id"])]
    xg_np = np.zeros((cfg.ngp, P), f16)
    xg_np[: cfg.ng] = (
        np.asarray(ii["game_x"], f32) @ np.asarray(ii["game_lin_w"], f32)
        + np.asarray(ii["game_lin_b"], f32)
        + np.asarray(ii["game_emb"], f32)[np.asarray(ii["game_node_id"])])

    key = (edge_src[:64].tobytes(), edge_dst[:64].tobytes(),
           int(edge_src.sum()), int(edge_dst.sum()))
    if key in _cache:
        ua, ua_t, ga, ga_t, lk, rcp, nc = _cache[key]
    else:
        ua, ua_t, ga, ga_t, lk, rcp = prep(cfg, edge_src, edge_dst,
                                           el_src, el_dst)
        nc = build(cfg, ua, ga, lk)
        _cache[key] = (ua, ua_t, ga, ga_t, lk, rcp, nc)

    wmap = {"wl1u": ii["w1l_r"], "wr1u": ii["w1r_r"],
            "wl1g": ii["w1l_o"], "wr1g": ii["w1r_o"],
            "wl2u": ii["w2l_r"], "wr2u": ii["w2r_r"],
            "wl2g": ii["w2l_o"], "wr2g": ii["w2r_o"]}
    bmap = {"b1u": ii["b1l_r"], "b1g": ii["b1l_o"],
            "b2u": ii["b2l_r"], "b2g": ii["b2l_o"]}
    in_maps = _make_inputs(cfg, xu_np, xg_np, wmap, bmap, ua_t, ga_t, lk,
                           rcp)

    res = run_bass_kernel_spmd(nc, in_maps,
                               core_ids=list(range(cfg.ncores)),
                               **(run_kwargs or {}))
    global LAST_RESULTS
    LAST_RESULTS = res

    result = np.zeros(cfg.el, f32)
    for c in range(cfg.ncores):
        oc = np.asarray(res.results[c]["out"], f32)   # [128, chunks]
        flat = oc.T.reshape(-1)                        # slot j = flat[j]
        valid = lk.orig[c] >= 0
        result[lk.orig[c][valid]] = flat[valid]
    return result


def kernel(user_node_id, game_node_id, game_x, edge_src, edge_dst,
           el_src, el_dst, user_emb, game_emb, game_lin_w, game_lin_b,
           w1l_o, b1l_o, w1r_o, w1l_r, b1l_r, w1r_r,
           w2l_o, b2l_o, w2r_o, w2l_r, b2l_r, w2r_r):
    cfg = CFG(8, 100000, 50000, 128, 1000000, 500000)
    return run(cfg, dict(
        user_node_id=user_node_id, game_node_id=game_node_id,
        game_x=game_x, edge_src=edge_src, edge_dst=edge_dst,
        el_src=el_src, el_dst=el_dst, user_emb=user_emb,
        game_emb=game_emb, game_lin_w=game_lin_w, game_lin_b=game_lin_b,
        w1l_o=w1l_o, b1l_o=b1l_o, w1r_o=w1r_o,
        w1l_r=w1l_r, b1l_r=b1l_r, w1r_r=w1r_r,
        w2l_o=w2l_o, b2l_o=b2l_o, w2r_o=w2r_o,
        w2l_r=w2l_r, b2l_r=b2l_r, w2r_r=w2r_r))

